# revision 1
# baseline (speedup 1.0000x reference)
"""Trainium2 Bass kernel: out = x @ ((W_int + offset) * scale).

Math: out[m,n] = scale[n] * (sum_k x[m,k]*W[k,n] + offset[n]*rowsum(x)[m])
so the dequantized weight is never materialized. The rank-1 offset term is
folded into the matmul as an extra 128-row K-block (row 0 = rowsum(x) /
offset, rest zeros), and scale is applied in the PSUM->SBUF epilogue.

Sharding: column-parallel — W / scale / offset / out split along N across
8 cores; x (as bf16 x^T) replicated.
"""

import numpy as np
import ml_dtypes

M, K, N = 4096, 4096, 11008
NCORES = 8
NSH = N // NCORES  # 1376
P = 128
KO = K // P        # 32
KOA = KO + 1       # 33: one extra 128-block for the rank-1 offset term
KA = KOA * P       # 4224
N_TILES = [(0, 512), (512, 512), (1024, 352)]

_BF16 = ml_dtypes.bfloat16

_cache = {}


def _build_nc():
    import concourse.bacc as bacc
    import concourse.mybir as mybir
    import concourse.tile as tile

    bf16 = mybir.dt.bfloat16
    f32 = mybir.dt.float32

    nc = bacc.Bacc(None, target_bir_lowering=False)
    xt = nc.dram_tensor("xt", [KA, M], bf16, kind="ExternalInput")
    w = nc.dram_tensor("w", [KA, NSH], bf16, kind="ExternalInput")
    scaleb = nc.dram_tensor("scaleb", [P, NSH], f32, kind="ExternalInput")
    out = nc.dram_tensor("out", [M, NSH], f32, kind="ExternalOutput")

    xt3 = xt.ap().rearrange("(ko p) m -> p ko m", p=P)    # [128, 33, 4096]
    w3 = w.ap().rearrange("(ko p) n -> p ko n", p=P)      # [128, 33, 1376]
    out3 = out.ap().rearrange("(mo p) n -> p mo n", p=P)  # [128, 32, 1376]

    with tile.TileContext(nc) as tc:
        with (
            tc.tile_pool(name="wpool", bufs=1) as wpool,
            tc.tile_pool(name="xpool", bufs=3) as xpool,
            tc.tile_pool(name="opool", bufs=3) as opool,
            tc.tile_pool(name="cpool", bufs=1) as cpool,
            tc.tile_pool(name="psp", bufs=8, space="PSUM") as psp,
        ):
            scale_sb = cpool.tile([P, NSH], f32, tag="scale")
            nc.sync.dma_start(scale_sb[:], scaleb[:])

            w_sb = []
            for ko in range(KOA):
                t = wpool.tile([P, NSH], bf16, tag=f"w{ko}")
                nc.sync.dma_start(t[:], w3[:, ko, :])
                w_sb.append(t)

            for mo in range(M // P):
                x_sb = xpool.tile([P, KOA, P], bf16, tag="x")
                nc.sync.dma_start(x_sb[:], xt3[:, :, mo * P:(mo + 1) * P])
                o_sb = opool.tile([P, NSH], f32, tag="o")
                for n0, nw in N_TILES:
                    ps = psp.tile([P, 512], f32, tag="ps")
                    for ko in range(KOA):
                        nc.tensor.matmul(
                            ps[:, :nw],
                            x_sb[:, ko, :],
                            w_sb[ko][:, n0:n0 + nw],
                            start=(ko == 0),
                            stop=(ko == KOA - 1),
                        )
                    nc.vector.tensor_mul(
                        out=o_sb[:, n0:n0 + nw],
                        in0=ps[:, :nw],
                        in1=scale_sb[:, n0:n0 + nw],
                    )
                nc.sync.dma_start(out3[:, mo, :], o_sb[:])
    nc.compile()
    return nc


def _get_nc():
    if "nc" not in _cache:
        _cache["nc"] = _build_nc()
    return _cache["nc"]


def _prep_inputs(x, weight, antiquant_scale, antiquant_offset):
    x = np.asarray(x, dtype=np.float32)
    weight = np.asarray(weight)
    antiquant_scale = np.asarray(antiquant_scale, dtype=np.float32)
    antiquant_offset = np.asarray(antiquant_offset, dtype=np.float32)

    # x^T (K x M) in bf16, plus one extra 128-row block whose row 0 is
    # rowsum(x); paired with a weight row of offset[n] it contributes
    # rowsum(x)[m] * offset[n] to the accumulation.
    xt_aug = np.zeros((KA, M), dtype=_BF16)
    xt_aug[:K] = x.astype(_BF16).T
    xt_aug[K] = x.sum(axis=1).astype(_BF16)

    in_maps = []
    for c in range(NCORES):
        sl = slice(c * NSH, (c + 1) * NSH)
        w_aug = np.zeros((KA, NSH), dtype=_BF16)
        w_aug[:K] = weight[:, sl].astype(_BF16)  # ints 0..126: exact in bf16
        w_aug[K] = antiquant_offset[sl].astype(_BF16)
        scaleb = np.ascontiguousarray(
            np.broadcast_to(antiquant_scale[sl][None, :], (P, NSH))
        )
        in_maps.append({"xt": xt_aug, "w": w_aug, "scaleb": scaleb})
    return in_maps


def kernel(x, weight, antiquant_scale, antiquant_offset, _trace=False):
    from concourse.bass_utils import run_bass_kernel_spmd

    nc = _get_nc()
    in_maps = _prep_inputs(x, weight, antiquant_scale, antiquant_offset)
    res = run_bass_kernel_spmd(
        nc, in_maps, core_ids=list(range(NCORES)), trace=_trace
    )
    out = np.concatenate([res.results[c]["out"] for c in range(NCORES)], axis=1)
    if _trace:
        _cache["last_result"] = res
    return out


# revision 2
# speedup vs baseline: 1.0506x; 1.0506x over previous
"""Trainium2 Bass kernel: out = x @ ((W_int + offset) * scale).

Math: out[m,n] = scale[n] * ((x @ W)[m,n] + offset[n] * rowsum(x)[m]),
so the dequantized weight is never materialized: plain bf16 matmul
(W ints 0..126 are exact in bf16) plus a rank-1 epilogue fused into two
vector-engine ops per output tile.

Sharding: column-parallel — W / scale / offset / out split along N across
8 cores; x (as bf16 x^T) replicated.

Per-core kernel: the whole W shard (11 MB bf16) is cached in SBUF, W-block
loads split across both HWDGE queues (sync + scalar); x^T m-tiles stream
through with double-buffering; PSUM accumulates over 32 K-blocks.
"""

import numpy as np
import ml_dtypes

M, K, N = 4096, 4096, 11008
NCORES = 8
NSH = N // NCORES  # 1376
P = 128
KO = K // P        # 32
MO = M // P        # 32
N_TILES = [(0, 512), (512, 512), (1024, 352)]

_BF16 = ml_dtypes.bfloat16

_cache = {}


def _build_nc():
    import concourse.bacc as bacc
    import concourse.mybir as mybir
    import concourse.tile as tile

    bf16 = mybir.dt.bfloat16
    f32 = mybir.dt.float32

    nc = bacc.Bacc(None, target_bir_lowering=False)
    xt = nc.dram_tensor("xt", [K, M], bf16, kind="ExternalInput")
    w = nc.dram_tensor("w", [K, NSH], bf16, kind="ExternalInput")
    scaleb = nc.dram_tensor("scaleb", [P, NSH], f32, kind="ExternalInput")
    offb = nc.dram_tensor("offb", [P, NSH], f32, kind="ExternalInput")
    scol = nc.dram_tensor("scol", [P, MO], f32, kind="ExternalInput")
    out = nc.dram_tensor("out", [M, NSH], f32, kind="ExternalOutput")

    xt3 = xt.ap().rearrange("(ko p) m -> p ko m", p=P)    # [128, 32, 4096]
    w3 = w.ap().rearrange("(ko p) n -> p ko n", p=P)      # [128, 32, 1376]
    out3 = out.ap().rearrange("(mo p) n -> p mo n", p=P)  # [128, 32, 1376]

    with tile.TileContext(nc) as tc:
        with (
            tc.tile_pool(name="wpool", bufs=1) as wpool,
            tc.tile_pool(name="xpool", bufs=4) as xpool,
            tc.tile_pool(name="opool", bufs=3) as opool,
            tc.tile_pool(name="cpool", bufs=1) as cpool,
            tc.tile_pool(name="psp", bufs=8, space="PSUM") as psp,
        ):
            x_tiles = {}

            def load_x(mo):
                t = xpool.tile([P, KO, P], bf16, tag="x")
                nc.sync.dma_start(t[:], xt3[:, :, mo * P:(mo + 1) * P])
                x_tiles[mo] = t

            # x m-tile 0 first so the PE can start ~immediately.
            load_x(0)

            scale_sb = cpool.tile([P, NSH], f32, tag="scale")
            nc.sync.dma_start(scale_sb[:], scaleb[:])
            off_sb = cpool.tile([P, NSH], f32, tag="off")
            nc.scalar.dma_start(off_sb[:], offb[:])
            scol_sb = cpool.tile([P, MO], f32, tag="scol")
            nc.scalar.dma_start(scol_sb[:], scol[:])

            # W shard: 32 blocks, alternating across the two HWDGE queues.
            w_sb = []
            for ko in range(KO):
                t = wpool.tile([P, NSH], bf16, tag=f"w{ko}")
                eng = nc.sync if ko % 2 == 0 else nc.scalar
                eng.dma_start(t[:], w3[:, ko, :])
                w_sb.append(t)

            for mo in range(MO):
                if mo + 1 < MO:
                    load_x(mo + 1)
                x_sb = x_tiles.pop(mo)
                o_sb = opool.tile([P, NSH], f32, tag="o")
                for n0, nw in N_TILES:
                    ps = psp.tile([P, 512], f32, tag="ps")
                    for ko in range(KO):
                        nc.tensor.matmul(
                            ps[:, :nw],
                            x_sb[:, ko, :],
                            w_sb[ko][:, n0:n0 + nw],
                            start=(ko == 0),
                            stop=(ko == KO - 1),
                        )
                    # ps += offset[n] * s[m]   (rank-1 term, fused DVE op)
                    nc.vector.scalar_tensor_tensor(
                        ps[:, :nw],
                        off_sb[:, n0:n0 + nw],
                        scol_sb[:, mo:mo + 1],
                        ps[:, :nw],
                        mybir.AluOpType.mult,
                        mybir.AluOpType.add,
                    )
                    # out = ps * scale[n]
                    nc.vector.tensor_mul(
                        out=o_sb[:, n0:n0 + nw],
                        in0=ps[:, :nw],
                        in1=scale_sb[:, n0:n0 + nw],
                    )
                nc.scalar.dma_start(out3[:, mo, :], o_sb[:])
    nc.compile()
    return nc


def _get_nc():
    if "nc" not in _cache:
        _cache["nc"] = _build_nc()
    return _cache["nc"]


def _prep_inputs(x, weight, antiquant_scale, antiquant_offset):
    x = np.asarray(x, dtype=np.float32)
    weight = np.asarray(weight)
    antiquant_scale = np.asarray(antiquant_scale, dtype=np.float32)
    antiquant_offset = np.asarray(antiquant_offset, dtype=np.float32)

    xt = np.ascontiguousarray(x.astype(_BF16).T)             # [K, M] bf16
    s = x.sum(axis=1, dtype=np.float32)                      # [M]
    scol = np.ascontiguousarray(s.reshape(MO, P).T)          # [P, MO]

    in_maps = []
    for c in range(NCORES):
        sl = slice(c * NSH, (c + 1) * NSH)
        wc = np.ascontiguousarray(weight[:, sl].astype(_BF16))
        scaleb = np.ascontiguousarray(
            np.broadcast_to(antiquant_scale[sl][None, :], (P, NSH))
        )
        offb = np.ascontiguousarray(
            np.broadcast_to(antiquant_offset[sl][None, :], (P, NSH))
        )
        in_maps.append(
            {"xt": xt, "w": wc, "scaleb": scaleb, "offb": offb, "scol": scol}
        )
    return in_maps


def kernel(x, weight, antiquant_scale, antiquant_offset, _trace=False):
    from concourse.bass_utils import run_bass_kernel_spmd

    nc = _get_nc()
    in_maps = _prep_inputs(x, weight, antiquant_scale, antiquant_offset)
    res = run_bass_kernel_spmd(
        nc, in_maps, core_ids=list(range(NCORES)), trace=_trace
    )
    out = np.concatenate([res.results[c]["out"] for c in range(NCORES)], axis=1)
    if _trace:
        _cache["last_result"] = res
    return out
